# revision 4
# baseline (speedup 1.0000x reference)
"""Sparse (block-diagonal) attention kernel for Trainium2, 8-core SPMD.

Reference computation (per query i in group g):
    qz = q @ Wq + bq                      (N, 256)
    kz = k @ Wk + bk                      (n, 128, 256)
    s[i, l] = <kz[g, l], qz[i]> / 16
    p = softmax(mask(s))
    out[i]  = sum_l p[i, l] * v[g, l]

Key algebraic transform (exact under softmax shift invariance):
    <k@Wk + bk, qz> = <k, Wk @ qz> + <bk, qz>
The <bk, qz> term is constant per query row, so it drops out of the
softmax.  We therefore compute u = (q@Wq + bq) @ Wk^T / 16 once (tiny)
and score directly against raw k — a ~10x FLOP reduction that makes the
kernel memory-bound (stream k and v once).

Sharding: groups (and their query slices) are split evenly across the 8
NeuronCores; the small projection weights are replicated.
"""

import os
from contextlib import ExitStack

import numpy as np

N_CORES = 8
N_GROUPS = 1024
L = 128              # keys per group
R = 4                # queries per group
D = 256              # d_q = d_k = d_z = d_v
G_CORE = N_GROUPS // N_CORES      # 128 groups per core
Q_CORE = G_CORE * R               # 512 queries per core
GB = 32                           # groups per compute block
NBLK = G_CORE // GB               # 4 blocks per core
QB = GB * R                       # 128 query columns per block
SCALE = 1.0 / 16.0                # 1/sqrt(d_z)
NEG = -1.0e30                     # additive mask value

_CACHE = {}


def _build_bass():
    import concourse.tile as tile
    from concourse import bacc, mybir
    from concourse.masks import make_identity

    f32 = mybir.dt.float32
    u8 = mybir.dt.uint8

    nc = bacc.Bacc(None, target_bir_lowering=False, debug=True)
    q = nc.dram_tensor("q", (Q_CORE, D), f32, kind="ExternalInput")
    k = nc.dram_tensor("k", (G_CORE, L, D), f32, kind="ExternalInput")
    v = nc.dram_tensor("v", (G_CORE, L, D), f32, kind="ExternalInput")
    m = nc.dram_tensor("m", (G_CORE, L), u8, kind="ExternalInput")
    wq = nc.dram_tensor("wq", (D, D), f32, kind="ExternalInput")
    wk = nc.dram_tensor("wk", (D, D), f32, kind="ExternalInput")
    bq = nc.dram_tensor("bq", (D,), f32, kind="ExternalInput")
    out = nc.dram_tensor("out", (Q_CORE, D), f32, kind="ExternalOutput")

    with tile.TileContext(nc) as tc, ExitStack() as ctx:
        singles = ctx.enter_context(tc.tile_pool(name="singles", bufs=1))

        identity = singles.tile([128, 128], f32)
        make_identity(nc, identity)
        ones = singles.tile([128, 1], f32)
        nc.vector.memset(ones, 1.0)

        # ---- replicated weights -------------------------------------
        wq_sb = singles.tile([128, 2, D], f32)   # [c_in, c_half, z]
        nc.sync.dma_start(wq_sb, wq[:].rearrange("(h p) z -> p h z", p=128))
        wk_sb = singles.tile([128, 2, D], f32)   # [dk_in, dk_half, z]
        nc.sync.dma_start(wk_sb, wk[:].rearrange("(h p) z -> p h z", p=128))
        bq_sb = singles.tile([128, 2], f32)      # [z_in, z_half]
        for h in range(2):
            nc.sync.dma_start(bq_sb[:, h : h + 1], bq[h * 128 : (h + 1) * 128][:, None])

        q_sb = singles.tile([128, 4, D], f32)    # [i_in, i_tile, c]
        nc.sync.dma_start(q_sb, q[:].rearrange("(t p) c -> p t c", p=128))

        qT_sb = singles.tile([128, 2, Q_CORE], f32)   # [c_in, c_half, i]
        qzT_sb = singles.tile([128, 2, Q_CORE], f32)  # [z_in, z_half, i]
        wkT_sb = singles.tile([128, 2, D], f32)       # [z_in, z_half, dk]  (scaled)
        ut_sb = singles.tile([128, 2, Q_CORE], f32)   # [d_in, d_half, i]

        # ---- preamble: u = (q@Wq + bq) @ Wk^T / 16, stored transposed
        with tc.tile_pool(name="pre_ps", bufs=2, space="PSUM") as pre_ps:
            # qT via PE transposes
            for ch in range(2):
                ps = pre_ps.tile([128, Q_CORE], f32, tag="pre")
                for t in range(4):
                    nc.tensor.transpose(
                        ps[:, t * 128 : (t + 1) * 128],
                        q_sb[:, t, ch * 128 : (ch + 1) * 128],
                        identity,
                    )
                nc.vector.tensor_copy(qT_sb[:, ch, :], ps)
            # wkT via PE transposes (scaled by 1/16 on copy-back)
            for zh in range(2):
                ps = pre_ps.tile([128, Q_CORE], f32, tag="pre")
                for dh in range(2):
                    nc.tensor.transpose(
                        ps[:, dh * 128 : (dh + 1) * 128],
                        wk_sb[:, dh, zh * 128 : (zh + 1) * 128],
                        identity,
                    )
                nc.vector.tensor_scalar_mul(wkT_sb[:, zh, :], ps[:, :D], SCALE)
            # qzT = Wq^T @ qT (+ bq)
            for zh in range(2):
                ps = pre_ps.tile([128, Q_CORE], f32, tag="pre")
                for ch in range(2):
                    nc.tensor.matmul(
                        ps,
                        lhsT=wq_sb[:, ch, zh * 128 : (zh + 1) * 128],
                        rhs=qT_sb[:, ch, :],
                        start=(ch == 0),
                        stop=(ch == 1),
                    )
                nc.vector.tensor_scalar(
                    qzT_sb[:, zh, :],
                    ps,
                    scalar1=bq_sb[:, zh : zh + 1],
                    scalar2=None,
                    op0=mybir.AluOpType.add,
                )
            # ut = (Wk/16) @ qzT
            for dh in range(2):
                ps = pre_ps.tile([128, Q_CORE], f32, tag="pre")
                for zh in range(2):
                    nc.tensor.matmul(
                        ps,
                        lhsT=wkT_sb[:, zh, dh * 128 : (dh + 1) * 128],
                        rhs=qzT_sb[:, zh, :],
                        start=(zh == 0),
                        stop=(zh == 1),
                    )
                nc.vector.tensor_copy(ut_sb[:, dh, :], ps)

        # ---- main pools ---------------------------------------------
        kv = ctx.enter_context(tc.tile_pool(name="kv", bufs=2))
        work = ctx.enter_context(tc.tile_pool(name="work", bufs=2))
        ktp = ctx.enter_context(tc.tile_pool(name="ktp", bufs=3))
        kt_ps = ctx.enter_context(tc.tile_pool(name="kt_ps", bufs=2, space="PSUM"))
        st_ps = ctx.enter_context(tc.tile_pool(name="st_ps", bufs=2, space="PSUM"))
        ot_ps = ctx.enter_context(tc.tile_pool(name="ot_ps", bufs=2, space="PSUM"))
        misc_ps = ctx.enter_context(tc.tile_pool(name="misc_ps", bufs=2, space="PSUM"))

        for b in range(NBLK):
            g0 = b * GB
            k_sb = kv.tile([128, GB, D], f32, tag="k_sb")  # [l, g, d]
            nc.sync.dma_start(k_sb, k[g0 : g0 + GB].rearrange("g l d -> l g d"))
            v_sb = kv.tile([128, GB, D], f32, tag="v_sb")  # [l, g, d]
            nc.sync.dma_start(v_sb, v[g0 : g0 + GB].rearrange("g l d -> l g d"))

            # mask -> additive form in transposed (keys x queries) layout
            m_sb = work.tile([GB, L], u8, tag="m_sb")
            nc.sync.dma_start(m_sb, m[g0 : g0 + GB, :])
            mf = work.tile([128, L], f32, tag="mf")
            nc.vector.memset(mf, 1.0)                        # pad rows: no masking
            nc.vector.tensor_copy(mf[:GB, :], m_sb)          # u8 -> f32 cast
            mt = misc_ps.tile([128, 128], f32, tag="misc")
            nc.tensor.transpose(mt, mf, identity)            # [l, g]
            madd = work.tile([128, QB], f32, tag="madd")
            nc.vector.tensor_scalar(
                madd.rearrange("p (g j) -> p g j", g=GB),
                mt[:, :GB, None].to_broadcast((128, GB, R)),
                scalar1=1.0,
                scalar2=-NEG,
                op0=mybir.AluOpType.subtract,
                op1=mybir.AluOpType.mult,
            )  # (m - 1) * 1e30  ->  0 for valid, -1e30 for masked

            # scores St[l, q] for the whole block
            st = st_ps.tile([128, QB], f32, tag="st")
            for gi in range(GB):
                kt_p = kt_ps.tile([128, 2, 128], f32, tag="kt")  # [d_in, dh, l]
                for dh in range(2):
                    nc.tensor.transpose(
                        kt_p[:, dh, :],
                        k_sb[:, gi, dh * 128 : (dh + 1) * 128],
                        identity,
                    )
                kt_sb = ktp.tile([128, 2, 128], f32, tag="kt_sb")
                if gi % 2 == 0:
                    nc.vector.tensor_copy(kt_sb, kt_p)
                else:
                    nc.scalar.copy(kt_sb, kt_p)
                qc = b * QB + gi * R
                for dh in range(2):
                    nc.tensor.matmul(
                        st[:, gi * R : (gi + 1) * R],
                        lhsT=kt_sb[:, dh, :],
                        rhs=ut_sb[:, dh, qc : qc + R],
                        start=(dh == 0),
                        stop=(dh == 1),
                    )

            # masked softmax over keys (partition dim)
            s_m = work.tile([128, QB], f32, tag="s_m")
            nc.vector.tensor_tensor(s_m, st, madd, mybir.AluOpType.add)
            pm = work.tile([128, QB], f32, tag="pm")
            nc.scalar.activation(pm, s_m, mybir.ActivationFunctionType.Exp)
            sums = misc_ps.tile([128, 128], f32, tag="misc")
            nc.tensor.matmul(sums[:, :1], lhsT=pm, rhs=ones, start=True, stop=True)
            rrec = work.tile([128, 1], f32, tag="rrec")
            nc.vector.reciprocal(rrec, sums[:, :1])

            # OT[dv, q] = v^T @ pm  (per group)
            ot = ot_ps.tile([128, 2, QB], f32, tag="ot")  # [dv_in, dvh, q]
            for gi in range(GB):
                for dvh in range(2):
                    nc.tensor.matmul(
                        ot[:, dvh, gi * R : (gi + 1) * R],
                        lhsT=v_sb[:, gi, dvh * 128 : (dvh + 1) * 128],
                        rhs=pm[:, gi * R : (gi + 1) * R],
                        start=True,
                        stop=True,
                    )

            # repack OT -> out rows (q, dv), normalized by 1/sums
            out_sb = work.tile([128, D], f32, tag="out_sb")
            for dvh in range(2):
                ot_sb = work.tile([128, QB], f32, tag="ot_sb")
                if dvh == 0:
                    nc.vector.tensor_copy(ot_sb, ot[:, dvh, :])
                else:
                    nc.scalar.copy(ot_sb, ot[:, dvh, :])
                o_t = misc_ps.tile([128, 128], f32, tag="misc")
                nc.tensor.transpose(o_t, ot_sb, identity)  # [q, dv]
                nc.vector.tensor_scalar_mul(
                    out_sb[:, dvh * 128 : (dvh + 1) * 128], o_t, rrec
                )
            nc.sync.dma_start(out[b * 128 : (b + 1) * 128, :], out_sb)

    nc.compile()
    return nc


def _get_nc():
    if "nc" not in _CACHE:
        _CACHE["nc"] = _build_bass()
    return _CACHE["nc"]


def _make_in_maps(inputs):
    q = np.ascontiguousarray(np.asarray(inputs["q"], dtype=np.float32))
    k = np.ascontiguousarray(np.asarray(inputs["k"], dtype=np.float32))
    v = np.ascontiguousarray(np.asarray(inputs["v"], dtype=np.float32))
    m = np.ascontiguousarray(np.asarray(inputs["m"]).astype(np.uint8))
    wq = np.ascontiguousarray(np.asarray(inputs["Wq"], dtype=np.float32))
    wk = np.ascontiguousarray(np.asarray(inputs["Wk"], dtype=np.float32))
    bq = np.ascontiguousarray(np.asarray(inputs["bq"], dtype=np.float32))

    in_maps = []
    for c in range(N_CORES):
        gs, ge = c * G_CORE, (c + 1) * G_CORE
        qs, qe = c * Q_CORE, (c + 1) * Q_CORE
        in_maps.append(
            {
                "q": q[qs:qe],
                "k": k[gs:ge],
                "v": v[gs:ge],
                "m": m[gs:ge],
                "wq": wq,
                "wk": wk,
                "bq": bq,
            }
        )
    return in_maps


def run(inputs, trace=False):
    """Run the SPMD kernel; returns (full_output, exec_time_ns_or_None)."""
    from concourse.bass_utils import run_bass_kernel_spmd

    nc = _get_nc()
    in_maps = _make_in_maps(inputs)
    res = run_bass_kernel_spmd(
        nc, in_maps, core_ids=list(range(N_CORES)), trace=trace
    )
    outs = [res.results[c]["out"] for c in range(N_CORES)]
    full = np.concatenate(outs, axis=0).astype(np.float32)
    return full, res.exec_time_ns


def kernel(**inputs) -> np.ndarray:
    full, _ = run(inputs, trace=False)
    return full


# revision 26
# speedup vs baseline: 1243.6317x; 1243.6317x over previous
"""Sparse (block-diagonal) attention kernel for Trainium2, 8-core SPMD.

Reference computation (per query i in group g):
    qz = q @ Wq + bq                      (N, 256)
    kz = k @ Wk + bk                      (n, 128, 256)
    s[i, l] = <kz[g, l], qz[i]> / 16
    p = softmax(mask(s))
    out[i]  = sum_l p[i, l] * v[g, l]

Key algebraic transform (exact under softmax shift invariance):
    <k@Wk + bk, qz> = <k, Wk @ qz> + <bk, qz>
The <bk, qz> term is constant per query row, so it drops out of the
softmax.  We therefore compute u = (q@Wq + bq) @ Wk^T / 16 once (tiny)
and score directly against raw k — a ~10x FLOP reduction that makes the
kernel memory-bound (stream k and v once).

Sharding: groups (and their query slices) are split evenly across the 8
NeuronCores; the small projection weights are replicated.
"""

import os
from contextlib import ExitStack

import numpy as np

N_CORES = 8
N_GROUPS = 1024
L = 128              # keys per group
R = 4                # queries per group
D = 256              # d_q = d_k = d_z = d_v
G_CORE = N_GROUPS // N_CORES      # 128 groups per core
Q_CORE = G_CORE * R               # 512 queries per core
GB = 16                           # groups per compute block
NBLK = G_CORE // GB               # 4 blocks per core
QB = GB * R                       # 128 query columns per block
SCALE = 1.0 / 16.0                # 1/sqrt(d_z)
NEG = -1.0e30                     # additive mask value

_CACHE = {}


def _build_bass():
    import concourse.tile as tile
    from concourse import bacc, mybir

    f32 = mybir.dt.float32
    u8 = mybir.dt.uint8

    nc = bacc.Bacc(None, target_bir_lowering=False, debug=True)
    q = nc.dram_tensor("q", (Q_CORE, D), f32, kind="ExternalInput")
    k = nc.dram_tensor("k", (G_CORE, L, D), f32, kind="ExternalInput")
    v = nc.dram_tensor("v", (G_CORE, L, D), f32, kind="ExternalInput")
    m = nc.dram_tensor("m", (G_CORE, L), u8, kind="ExternalInput")
    wq = nc.dram_tensor("wq", (D, D), f32, kind="ExternalInput")
    wk = nc.dram_tensor("wk", (D, D), f32, kind="ExternalInput")
    bq = nc.dram_tensor("bq", (D,), f32, kind="ExternalInput")
    ident = nc.dram_tensor("ident", (128, 128), f32, kind="ExternalInput")
    out = nc.dram_tensor("out", (Q_CORE, D), f32, kind="ExternalOutput")

    with tile.TileContext(nc) as tc, ExitStack() as ctx:
        singles = ctx.enter_context(tc.tile_pool(name="singles", bufs=1))

        # q first on the ACT ring: the whole preamble chain hangs off it
        q_sb = singles.tile([128, 4, D], f32)    # [i_in, i_tile, c]
        nc.scalar.dma_start(q_sb, q[:].rearrange("(t p) c -> p t c", p=128))

        identity = singles.tile([128, 128], f32)
        nc.scalar.dma_start(identity, ident[:])
        ones = singles.tile([128, 1], f32)
        nc.vector.memset(ones, 1.0)

        # ---- replicated weights -------------------------------------
        wq_sb = singles.tile([128, 2, D], f32)   # [c_in, c_half, z]
        nc.scalar.dma_start(wq_sb, wq[:].rearrange("(h p) z -> p h z", p=128))
        wk_sb = singles.tile([128, 2, D], f32)   # [dk_in, dk_half, z]
        nc.scalar.dma_start(wk_sb, wk[:].rearrange("(h p) z -> p h z", p=128))
        bq_sb = singles.tile([128, 2], f32)      # [z_in, z_half]
        for h in range(2):
            nc.scalar.dma_start(bq_sb[:, h : h + 1], bq[h * 128 : (h + 1) * 128][:, None])

        f32r = mybir.dt.float32r
        qT_sb = singles.tile([128, 2, Q_CORE], f32r)  # [c_in, c_half, i]
        qzT_sb = singles.tile([128, 2, Q_CORE], f32r)  # [z_in, z_half, i]
        wkT_sb = singles.tile([128, 2, D], f32r)      # [z_in, z_half, dk]  (scaled)
        wq_r = singles.tile([128, 2, D], f32r)        # rounded copy of wq
        ut_sb = singles.tile([128, 2, Q_CORE], f32)   # [d_in, d_half, i]

        # whole-core mask, transposed once: mT[l, g] (G_CORE == 128)
        m_all = singles.tile([128, L], u8)
        nc.scalar.dma_start(m_all, m[:, :])
        m_f = singles.tile([128, L], f32)
        nc.vector.tensor_copy(m_f, m_all)
        mT_sb = singles.tile([128, G_CORE], f32)

        # ---- preamble: u = (q@Wq + bq) @ Wk^T / 16, stored transposed
        with tc.tile_pool(name="pre_ps", bufs=2, space="PSUM") as pre_ps:
            mps = pre_ps.tile([128, Q_CORE], f32, tag="pre")
            nc.tensor.transpose(mps[:, :128], m_f, identity)
            nc.vector.tensor_copy(mT_sb, mps[:, :128])
            # qT via PE transposes
            for ch in range(2):
                ps = pre_ps.tile([128, Q_CORE], f32, tag="pre")
                for t in range(4):
                    nc.tensor.transpose(
                        ps[:, t * 128 : (t + 1) * 128],
                        q_sb[:, t, ch * 128 : (ch + 1) * 128],
                        identity,
                    )
                nc.vector.tensor_copy(qT_sb[:, ch, :], ps)
            # wkT via PE transposes (scaled by 1/16 on copy-back)
            for zh in range(2):
                ps = pre_ps.tile([128, Q_CORE], f32, tag="pre")
                for dh in range(2):
                    nc.tensor.transpose(
                        ps[:, dh * 128 : (dh + 1) * 128],
                        wk_sb[:, dh, zh * 128 : (zh + 1) * 128],
                        identity,
                    )
                nc.vector.tensor_scalar_mul(wkT_sb[:, zh, :], ps[:, :D], SCALE)
            # qzT = Wq^T @ qT (+ bq)   [float32r matmuls: 1-pass PE]
            nc.scalar.copy(wq_r, wq_sb)  # round to f32r
            for zh in range(2):
                ps = pre_ps.tile([128, Q_CORE], f32, tag="pre")
                for ch in range(2):
                    nc.tensor.matmul(
                        ps,
                        lhsT=wq_r[:, ch, zh * 128 : (zh + 1) * 128],
                        rhs=qT_sb[:, ch, :],
                        start=(ch == 0),
                        stop=(ch == 1),
                    )
                nc.vector.tensor_scalar(
                    qzT_sb[:, zh, :],
                    ps,
                    scalar1=bq_sb[:, zh : zh + 1],
                    scalar2=None,
                    op0=mybir.AluOpType.add,
                )
            # ut = (Wk/16) @ qzT
            for dh in range(2):
                ps = pre_ps.tile([128, Q_CORE], f32, tag="pre")
                for zh in range(2):
                    nc.tensor.matmul(
                        ps,
                        lhsT=wkT_sb[:, zh, dh * 128 : (dh + 1) * 128],
                        rhs=qzT_sb[:, zh, :],
                        start=(zh == 0),
                        stop=(zh == 1),
                    )
                nc.vector.tensor_copy(ut_sb[:, dh, :], ps)

        # ---- main pools ---------------------------------------------
        kp = ctx.enter_context(tc.tile_pool(name="kp", bufs=4))
        vp = ctx.enter_context(tc.tile_pool(name="vp", bufs=4))
        work = ctx.enter_context(tc.tile_pool(name="work", bufs=2))
        ktp = ctx.enter_context(tc.tile_pool(name="ktp", bufs=3))
        kt_ps = ctx.enter_context(tc.tile_pool(name="kt_ps", bufs=3, space="PSUM"))
        st_ps = ctx.enter_context(tc.tile_pool(name="st_ps", bufs=2, space="PSUM"))
        ot_ps = ctx.enter_context(tc.tile_pool(name="ot_ps", bufs=1, space="PSUM"))
        misc_ps = ctx.enter_context(tc.tile_pool(name="misc_ps", bufs=2, space="PSUM"))

        for b in range(NBLK):
            g0 = b * GB
            k_sb = kp.tile([128, GB, D], f32, tag="k_sb")  # [l, g, d]
            for s in range(4):
                gq = GB // 4
                nc.sync.dma_start(
                    k_sb[:, s * gq : (s + 1) * gq, :],
                    k[g0 + s * gq : g0 + (s + 1) * gq].rearrange("g l d -> l g d"),
                )
            v_sb = vp.tile([128, GB, D], f32, tag="v_sb")  # [l, g, d]
            for s in range(2):
                gh = GB // 2
                nc.gpsimd.dma_start(
                    v_sb[:, s * gh : (s + 1) * gh, :],
                    v[g0 + s * gh : g0 + (s + 1) * gh].rearrange("g l d -> l g d"),
                )

            # additive mask in (keys x queries) layout, from preloaded mT
            madd = work.tile([128, QB], f32, tag="madd")
            nc.vector.tensor_scalar(
                madd.rearrange("p (g j) -> p g j", g=GB),
                mT_sb[:, g0 : g0 + GB, None].to_broadcast((128, GB, R)),
                scalar1=1.0,
                scalar2=-NEG,
                op0=mybir.AluOpType.subtract,
                op1=mybir.AluOpType.mult,
            )  # (m - 1) * 1e30  ->  0 for valid, -1e30 for masked

            # scores St[l, q] for the whole block; kT staged 2 groups per
            # PSUM bank so the copy-back is one wide (128x512) op per pair
            st = st_ps.tile([128, QB], f32, tag="st")
            for pair in range(GB // 2):
                kt_p = kt_ps.tile([128, 2, 2, 128], f32, tag="kt")  # [d, gi2, dh, l]
                for gi2 in range(2):
                    gi = pair * 2 + gi2
                    for dh in range(2):
                        nc.tensor.transpose(
                            kt_p[:, gi2, dh, :],
                            k_sb[:, gi, dh * 128 : (dh + 1) * 128],
                            identity,
                        )
                kt_sb = ktp.tile([128, 2, 2, 128], f32, tag="kt_sb")
                nc.vector.tensor_copy(kt_sb[:, 0], kt_p[:, 0])
                nc.scalar.copy(kt_sb[:, 1], kt_p[:, 1])
                for gi2 in range(2):
                    gi = pair * 2 + gi2
                    qc = b * QB + gi * R
                    for dh in range(2):
                        nc.tensor.matmul(
                            st[:, gi * R : (gi + 1) * R],
                            lhsT=kt_sb[:, gi2, dh, :],
                            rhs=ut_sb[:, dh, qc : qc + R],
                            start=(dh == 0),
                            stop=(dh == 1),
                        )

            # masked softmax over keys (partition dim)
            s_m = work.tile([128, QB], f32, tag="s_m")
            nc.vector.tensor_tensor(s_m, st, madd, mybir.AluOpType.add)
            pm = work.tile([128, QB], f32, tag="pm")
            nc.scalar.activation(pm, s_m, mybir.ActivationFunctionType.Exp)
            sums = misc_ps.tile([128, 128], f32, tag="misc")
            nc.tensor.matmul(
                sums[:QB, :1], lhsT=pm, rhs=ones, start=True, stop=True
            )
            rrec = work.tile([128, 1], f32, tag="rrec")
            nc.vector.reciprocal(rrec[:QB], sums[:QB, :1])

            # OT[dv, q] = v^T @ pm  (per group)
            ot = ot_ps.tile([128, 2, QB], f32, tag="ot")  # [dv_in, dvh, q]
            for gi in range(GB):
                for dvh in range(2):
                    nc.tensor.matmul(
                        ot[:, dvh, gi * R : (gi + 1) * R],
                        lhsT=v_sb[:, gi, dvh * 128 : (dvh + 1) * 128],
                        rhs=pm[:, gi * R : (gi + 1) * R],
                        start=True,
                        stop=True,
                    )

            # repack OT -> out rows (q, dv), normalized by 1/sums
            out_sb = work.tile([128, D], f32, tag="out_sb")
            for dvh in range(2):
                ot_sb = work.tile([128, QB], f32, tag="ot_sb")
                if dvh == 0:
                    nc.vector.tensor_copy(ot_sb, ot[:, dvh, :])
                else:
                    nc.scalar.copy(ot_sb, ot[:, dvh, :])
                o_t = misc_ps.tile([128, 128], f32, tag="misc")
                nc.tensor.transpose(o_t[:QB, :], ot_sb, identity)  # [q, dv]
                nc.vector.tensor_scalar_mul(
                    out_sb[:QB, dvh * 128 : (dvh + 1) * 128], o_t[:QB, :], rrec[:QB]
                )
            nc.scalar.dma_start(out[b * QB : (b + 1) * QB, :], out_sb[:QB, :])

    nc.compile()
    return nc


def _get_nc():
    if "nc" not in _CACHE:
        _CACHE["nc"] = _build_bass()
    return _CACHE["nc"]


def _make_in_maps(inputs):
    q = np.ascontiguousarray(np.asarray(inputs["q"], dtype=np.float32))
    k = np.ascontiguousarray(np.asarray(inputs["k"], dtype=np.float32))
    v = np.ascontiguousarray(np.asarray(inputs["v"], dtype=np.float32))
    m = np.ascontiguousarray(np.asarray(inputs["m"]).astype(np.uint8))
    wq = np.ascontiguousarray(np.asarray(inputs["Wq"], dtype=np.float32))
    wk = np.ascontiguousarray(np.asarray(inputs["Wk"], dtype=np.float32))
    bq = np.ascontiguousarray(np.asarray(inputs["bq"], dtype=np.float32))
    ident = np.eye(128, dtype=np.float32)

    in_maps = []
    for c in range(N_CORES):
        gs, ge = c * G_CORE, (c + 1) * G_CORE
        qs, qe = c * Q_CORE, (c + 1) * Q_CORE
        in_maps.append(
            {
                "q": q[qs:qe],
                "k": k[gs:ge],
                "v": v[gs:ge],
                "m": m[gs:ge],
                "wq": wq,
                "wk": wk,
                "bq": bq,
                "ident": ident,
            }
        )
    return in_maps


def run(inputs, trace=False):
    """Run the SPMD kernel; returns (full_output, exec_time_ns_or_None)."""
    from concourse.bass_utils import run_bass_kernel_spmd

    nc = _get_nc()
    in_maps = _make_in_maps(inputs)
    res = run_bass_kernel_spmd(
        nc, in_maps, core_ids=list(range(N_CORES)), trace=trace
    )
    outs = [res.results[c]["out"] for c in range(N_CORES)]
    full = np.concatenate(outs, axis=0).astype(np.float32)
    return full, res.exec_time_ns


def kernel(**inputs) -> np.ndarray:
    full, _ = run(inputs, trace=False)
    return full
